# revision 3
# baseline (speedup 1.0000x reference)
"""Trainium2 Bass kernel for CapsuleLayer dynamic routing.

Problem: u = einsum('bpe,pjed->bpjd', inp, W[0]) + b, then 3 routing
iterations (softmax over j, weighted sum over p, squash) -> vj [B,J,D].

Shapes: B=16, P=1024, J=32, Dp=D=64.  W is 512MB fp32 -> DMA dominated.

Strategy (8 NeuronCores):
 - Shard P across cores: 128 p's per core; all batches on every core.
 - Host packs W (bf16) in 2MiB blocks of 4 pairs: block g holds pairs
   4g..4g+3, each pair a [128=(2p x 64e), JD=(d,j)] panel.  lhsT[pair] =
   block-diag([inp_pe, inp_po]) so one matmul computes u for 2 capsules
   with full contraction rows, M=32; 4 pairs run concurrently in the 4
   PE column groups.
 - Routing on-device: agreement = per-g flat DVE mul (2x mode) + tree
   reduce; softmax without max subtraction; weighted p-sum via 0/1-mask
   (Delta) matmuls col-group-tiled 4-wide into PSUM accumulators at
   partition offsets 0/32/64/96; AllReduce (fp32) of the per-core sums.
 - AR plumbing: warmup AR triggered at t~0 absorbs the one-time CC
   barrier+ramp; s0 is split in two pieces (g<8 ships mid-phase-1);
   result fetches are SWDGE casts to bf16 ordered after all triggers so
   the gpsimd FIFO never blocks a doorbell.
 - Squash decoupled: the eye-matmul broadcasts RAW s (PSUM-accumulating
   the two s0 pieces), the squash scale f = scale^2*sqrt(t)/(1+scale^2 t)
   is computed concurrently (ACT square + halvings) and applied to the
   agreement output instead of to v.  Final iteration's partials summed
   + squashed on host.
"""

import numpy as np
import ml_dtypes

import concourse.bass as bass
import concourse.tile as tile
from concourse import bacc, mybir
from concourse.bass_utils import run_bass_kernel_spmd

F32 = mybir.dt.float32
BF16 = mybir.dt.bfloat16
AX = mybir.AxisListType
AF = mybir.ActivationFunctionType

B = 16      # batch
J = 32      # output capsules
D = 64      # output capsule dim
E = 64      # input capsule dim
JD = J * D  # 2048


def build_program(n_cores: int, n_groups: int):
    """Build the SPMD Bass program. Per core: P_loc = 8*n_groups capsules."""
    G = n_groups
    ploc = 8 * G
    npair = ploc // 2
    nblk = npair // 4          # 2MiB DMA blocks of 4 pairs (= 1 group)
    UFREE = G * JD             # u free elements per partition
    GS0 = G // 2               # s0 split point (early AR over g < GS0)

    nc = bacc.Bacc("TRN2", target_bir_lowering=False, debug=False,
                   num_devices=n_cores)

    w_dram = nc.dram_tensor("w", [nblk, 128, 4 * JD], BF16, kind="ExternalInput")
    x_dram = nc.dram_tensor("x", [128, npair * 32], BF16, kind="ExternalInput")
    out_dram = nc.dram_tensor("out", [16, JD], F32, kind="ExternalOutput")

    with tile.TileContext(nc) as tc:
        with (
            tc.tile_pool(name="const", bufs=1) as constp,
            tc.tile_pool(name="wpool", bufs=2) as wpool,
            tc.tile_pool(name="upool", bufs=1) as upool,
            tc.tile_pool(name="work", bufs=2) as work,
            tc.tile_pool(name="small", bufs=1) as small,
            tc.tile_pool(name="pmain", bufs=2, space="PSUM") as pmain,
            tc.tile_pool(name="pacc", bufs=2, space="PSUM") as pacc,
            tc.tile_pool(name="dram", bufs=1, space="DRAM") as dramp,
        ):
            # ---- W tile for group 0 first (start streaming ASAP) ----
            wdma_engines = [nc.sync, nc.scalar]
            wtiles = {}

            def load_w(g):
                wt = wpool.tile([128, 4 * JD], BF16, tag="w",
                                name=f"wt{g}", bufs=3)
                wdma_engines[g % 2].dma_start(wt[:], w_dram[g])
                return wt

            wtiles[0] = load_w(0)
            wtiles[1] = load_w(1)

            # ---- static inputs -> SBUF ----
            x_sb = constp.tile([128, npair * 32], BF16)
            nc.sync.dma_start(x_sb[:], x_dram[:])
            # 0/1 mask constants built on-device (no DMA dependency):
            # delta[q, m] = (q % 16 == m); eye[r, q] = (q % 16 == r)
            I32 = mybir.dt.int32
            delta_sb = constp.tile([128, 16], BF16)
            qi = constp.tile([128, 128], I32)
            mi = constp.tile([128, 128], I32)
            ei = constp.tile([128, 128], F32)
            nc.gpsimd.iota(qi[:, :16], pattern=[[0, 16]], base=0,
                           channel_multiplier=1)
            nc.vector.tensor_scalar(qi[:, :16], qi[:, :16], 15, None,
                                    op0=mybir.AluOpType.bitwise_and)
            nc.gpsimd.iota(mi[:, :16], pattern=[[1, 16]], base=0,
                           channel_multiplier=0)
            nc.vector.tensor_tensor(ei[:, :16], qi[:, :16], mi[:, :16],
                                    op=mybir.AluOpType.is_equal)
            nc.vector.tensor_copy(delta_sb[:], ei[:, :16])
            eye_sb = constp.tile([16, 128], BF16)
            nc.gpsimd.iota(qi[:16, :], pattern=[[1, 128]], base=0,
                           channel_multiplier=0)
            nc.vector.tensor_scalar(qi[:16, :], qi[:16, :], 15, None,
                                    op0=mybir.AluOpType.bitwise_and)
            nc.gpsimd.iota(mi[:16, :], pattern=[[0, 128]], base=0,
                           channel_multiplier=1)
            nc.vector.tensor_tensor(ei[:16, :], qi[:16, :], mi[:16, :],
                                    op=mybir.AluOpType.is_equal)
            nc.vector.tensor_copy(eye_sb[:], ei[:16, :])

            u_sb = upool.tile([128, UFREE], BF16)

            # ---- collective helpers ----
            n_cc = [0]

            def trigger_ar(src_f32_dma_done, nelem=JD):
                """Issue the collective for a prepared cin; return cout."""
                i = n_cc[0]
                n_cc[0] += 1
                cin, shape = src_f32_dma_done
                cout = dramp.tile(shape, F32, tag=f"cout{i}",
                                  addr_space="Shared" if n_cores > 4 else "Local",
                                  name=f"cc_out{i}")
                nc.gpsimd.collective_compute(
                    "AllReduce", mybir.AluOpType.add,
                    replica_groups=[list(range(n_cores))],
                    ins=[cin.opt()], outs=[cout.opt()],
                )
                return cout

            # tiny warmup collective triggered FIRST thing: burns the
            # one-time CC barrier + ramp early so real ARs run at steady
            # rate while phase 1 still streams W.
            warm = small.tile([16, 16], F32, tag="warm", name="warm")
            nc.vector.memset(warm[:], 0.0)
            cin_w = dramp.tile([16, 16], F32, tag="cin_w", name="cc_in_w")
            nc.gpsimd.dma_start(cin_w[:], warm[:])
            trigger_ar((cin_w, [16, 16]), nelem=16)

            # ---- phase 1: stream W, matmul u, evict, accumulate s0 ----
            # s0 accumulated in two psum pieces split at GS0 so the first
            # (ramp-paying) AllReduce hides under phase 1.  Delta matmuls
            # are col-group tiled: ns-chunk c accumulates at partition
            # offset 32c (4 concurrent PE column groups).
            state = {"ps": pacc.tile([128, 512], F32, tag="pacc", name="ps0_0"),
                     "started": set()}
            pend = []          # u_sb slabs whose delta matmuls are pending
            couts = []         # AR output dram tiles, in trigger order

            def flush_delta(draining):
                off, hn = pend.pop(0)
                ps = state["ps"]
                for t in range(2):
                    c = 2 * hn + t
                    start = c not in state["started"]
                    state["started"].add(c)
                    stop = draining and not any(h == hn for _, h in pend)
                    nc.tensor.matmul(
                        ps[32 * c:32 * c + 16, :],
                        delta_sb[:],
                        u_sb[:, off + t * 512: off + (t + 1) * 512],
                        tile_position=(0, 32 * c),
                        start=start, stop=stop,
                        skip_group_check=True,
                    )

            def ship_piece(idx):
                # drain pending slabs into this piece and ship its AllReduce
                while pend:
                    flush_delta(True)
                ps = state["ps"]
                s_loc = small.tile([128, 512], F32, tag="s_loc",
                                   name=f"sloc{idx}", bufs=2)
                cin = dramp.tile([16, JD], F32, tag=f"cin{idx}",
                                 name=f"cc_in{idx}")
                for c in range(4):
                    if c % 2 == 0:
                        nc.scalar.copy(s_loc[32 * c:32 * c + 16, :],
                                       ps[32 * c:32 * c + 16, :])
                    else:
                        nc.vector.tensor_copy(s_loc[32 * c:32 * c + 16, :],
                                              ps[32 * c:32 * c + 16, :])
                    nc.gpsimd.dma_start(cin[:, c * 512:(c + 1) * 512],
                                        s_loc[32 * c:32 * c + 16, :])
                couts.append(trigger_ar((cin, [16, JD])))
                state["started"] = set()

            for g in range(G):
                if g >= 1 and g + 1 < G:
                    wtiles[g + 1] = load_w(g + 1)
                if g == GS0:
                    ship_piece(0)
                    state["ps"] = pacc.tile([128, 512], F32, tag="pacc",
                                            name="ps0_1")
                wt = wtiles.pop(g)
                for hn in range(2):
                    pm = pmain.tile([128, 1024], F32, tag="pmain",
                                    name=f"pm{g}_{hn}")
                    # cg outer / ns inner: consecutive matmuls share lhsT so
                    # the weight load can be deduplicated.
                    for cg in range(4):
                        pi = 4 * g + cg
                        lhsT = x_sb[:, pi * 32:(pi + 1) * 32]
                        base = cg * JD + hn * 1024
                        for ns in range(2):
                            nc.tensor.matmul(
                                pm[32 * cg:32 * cg + 32, ns * 512:(ns + 1) * 512],
                                lhsT,
                                wt[:, base + ns * 512: base + (ns + 1) * 512],
                                tile_position=(0, 32 * cg),
                            )
                    off = g * JD + hn * 1024
                    # eviction on DVE (idle in phase 1) so the HWDGE queues
                    # only stream W at full rate
                    nc.vector.tensor_copy(u_sb[:, off:off + 1024], pm[:])
                    pend.append((off, hn))
                    if len(pend) > 2:
                        flush_delta(False)
            ship_piece(1)

            # fetch AR results AFTER all phase-1 triggers are enqueued so
            # the gpsimd FIFO never blocks a doorbell; SWDGE casts to bf16
            # during the fetch.
            def fetch_ar(cout, idx):
                dst = small.tile([16, JD], BF16, tag=f"ar{idx % 2}",
                                 name=f"ar_dst{idx}")
                nc.gpsimd.dma_start(dst[:], cout[:])
                return dst

            s0_a = fetch_ar(couts[0], 0)
            s0_b = fetch_ar(couts[1], 1)

            # ---- broadcast raw s to all 128 partitions (bf16) ----
            v_sb = constp.tile([128, JD], BF16)

            def broadcast_s(pieces):
                # v_sb[(k,b), col] = sum(piece[b, col] for piece in pieces)
                for hn in range(2):
                    pv = pmain.tile([128, 1024], F32, tag="pmain", name=f"pv{hn}")
                    for t in range(2):
                        for pi, piece in enumerate(pieces):
                            nc.tensor.matmul(
                                pv[:, t * 512:(t + 1) * 512], eye_sb[:],
                                piece[:, hn * 1024 + t * 512: hn * 1024 + (t + 1) * 512],
                                start=(pi == 0), stop=(pi == len(pieces) - 1),
                                skip_group_check=True,
                            )
                    nc.scalar.copy(v_sb[:, hn * 1024:(hn + 1) * 1024], pv[:])

            # ---- squash factor f = scale^2*sqrt(t) / (1 + scale^2*t) ----
            # computed from the RAW broadcast s in 128-partition space,
            # concurrent with the agreement muls (ACT + small DVE ops).
            def f_chain(scale, it):
                s2v = small.tile([128, JD], BF16, tag="s2v", name=f"s2v{it}")
                nc.vector.tensor_mul(s2v[:], v_sb[:], v_sb[:])
                n = JD // 2
                while n >= J * 2:
                    nc.vector.tensor_add(s2v[:, :n], s2v[:, :n], s2v[:, n:2 * n])
                    n //= 2
                t = small.tile([128, J], F32, tag="t", name=f"t{it}")
                nc.vector.tensor_add(t[:], s2v[:, :J], s2v[:, J:2 * J])
                st = small.tile([128, J], F32, tag="st", name=f"st{it}")
                nc.scalar.sqrt(st[:], t[:])
                den = small.tile([128, J], F32, tag="den", name=f"den{it}")
                nc.vector.tensor_scalar(den[:], t[:], scale * scale, 1.0,
                                        op0=mybir.AluOpType.mult,
                                        op1=mybir.AluOpType.add)
                rec = small.tile([128, J], F32, tag="rec", name=f"rec{it}")
                nc.vector.reciprocal(rec[:], den[:])
                f = small.tile([128, J], F32, tag="f", name=f"f{it}")
                nc.vector.scalar_tensor_tensor(f[:], st[:], scale * scale, rec[:],
                                               op0=mybir.AluOpType.mult,
                                               op1=mybir.AluOpType.mult)
                return f

            broadcast_s([s0_a, s0_b])

            # ---- routing iterations ----
            # Work split: DVE owns groups 0..12 (3 chunks); GpSimd
            # (measured ~0.31x of DVE throughput) owns groups 13-15.
            braw = constp.tile([128, G * J], F32)
            bij = constp.tile([128, G * J], F32)

            CHUNKS = [(0, 4, nc.vector), (4, 4, nc.vector), (8, 4, nc.vector),
                      (12, 4, nc.gpsimd)]

            def agree_chunk(eng, g0, gc, b_h, tag):
                """tmp = u*v for groups [g0, g0+gc); in-place tree-reduce over
                d; write the agreement into b_h [128, gc*J] fp32."""
                ch = gc * JD
                u_ch = u_sb[:, g0 * JD:(g0 + gc) * JD]
                tmp = work.tile([128, ch], BF16, tag=f"tmp{tag}", name="tmp",
                                bufs=2 if tag == "v" else 1)
                eng.tensor_mul(
                    tmp[:].rearrange("p (g q) -> p g q", g=gc),
                    u_ch.rearrange("p (g q) -> p g q", g=gc),
                    v_sb[:].unsqueeze(1).broadcast_to([128, gc, JD]),
                )
                # pairwise add-tree over d, fully in-place inside tmp
                # (out == src0 range exactly at every level)
                r3 = tmp[:].rearrange("p (g q) -> p g q", g=gc)
                dl = D
                while dl > 2:
                    half = dl // 2 * J
                    eng.tensor_add(
                        r3[:, :, 0:half], r3[:, :, 0:half], r3[:, :, half:2 * half])
                    dl //= 2
                eng.tensor_add(
                    b_h.rearrange("p (g j) -> p g j", g=gc),
                    r3[:, :, 0:J], r3[:, :, J:2 * J])

            for it in (1, 2):
                scale = (1.0 / J) if it == 1 else 1.0
                for g0, gc, eng in CHUNKS:
                    tag = "g" if eng is nc.gpsimd else "v"
                    b_h = braw[:, g0 * J:(g0 + gc) * J]
                    agree_chunk(eng, g0, gc, b_h, tag=tag)
                f = f_chain(scale, it)
                fbc = f[:].unsqueeze(1).broadcast_to([128, G, J])
                braw3 = braw[:].rearrange("p (g j) -> p g j", g=G)
                if it == 1:
                    nc.vector.tensor_mul(
                        bij[:].rearrange("p (g j) -> p g j", g=G), braw3, fbc)
                else:
                    nc.vector.tensor_mul(braw3, braw3, fbc)
                    nc.vector.tensor_add(bij[:], bij[:], braw[:])
                # softmax over j (no max subtraction: |bij| is bounded small)
                eh = small.tile([128, G * J], F32, tag="eh", name="eh")
                nc.scalar.activation(eh[:], bij[:], AF.Exp)
                eh3 = eh[:].rearrange("p (g j) -> p g j", g=G)
                se = small.tile([128, G], F32, tag="se", name="se")
                nc.vector.reduce_sum(se[:], eh3, axis=AX.X)
                re = small.tile([128, G], F32, tag="re", name="re")
                nc.vector.reciprocal(re[:], se[:])
                c_full = small.tile([128, G * J], BF16, tag="c_h", name="c_full")
                nc.vector.tensor_mul(
                    c_full[:].rearrange("p (g j) -> p g j", g=G), eh3,
                    re[:].unsqueeze(2).broadcast_to([128, G, J]))
                ps = pacc.tile([128, 512], F32, tag="pacc", name=f"ps_it{it}")
                for g0, gc, eng in CHUNKS:
                    tag = "g" if eng is nc.gpsimd else "v"
                    ch = gc * JD
                    u_ch = u_sb[:, g0 * JD:(g0 + gc) * JD]
                    u4 = u_ch.rearrange("p (g d j) -> p g d j", g=gc, d=D)
                    # cu = u * c (c broadcast over middle d)
                    cu = work.tile([128, ch], BF16, tag=f"tmp{tag}", name="cu",
                                   bufs=2 if tag == "v" else 1)
                    eng.tensor_mul(
                        cu[:].rearrange("p (g d j) -> p g d j", g=gc, d=D),
                        u4,
                        c_full[:, g0 * J:(g0 + gc) * J]
                            .rearrange("p (g j) -> p g j", g=gc)
                            .unsqueeze(2).broadcast_to([128, gc, D, J]),
                    )
                    # s += sum_k cu  (Delta matmuls, col-group tiled 4-wide)
                    for gg in range(gc):
                        for c in range(4):
                            nc.tensor.matmul(
                                ps[32 * c:32 * c + 16, :],
                                delta_sb[:],
                                cu[:, gg * JD + c * 512: gg * JD + (c + 1) * 512],
                                tile_position=(0, 32 * c),
                                start=(g0 == 0 and gg == 0),
                                stop=(g0 == 12 and gg == gc - 1),
                                skip_group_check=True,
                            )
                if it == 1:
                    s_loc = small.tile([128, 512], F32, tag="s_loc",
                                       name="sloc_it1", bufs=2)
                    cin = dramp.tile([16, JD], F32, tag="cin_s1", name="cc_in_s1")
                    for c in range(4):
                        if c % 2 == 0:
                            nc.scalar.copy(s_loc[32 * c:32 * c + 16, :],
                                           ps[32 * c:32 * c + 16, :])
                        else:
                            nc.vector.tensor_copy(s_loc[32 * c:32 * c + 16, :],
                                                  ps[32 * c:32 * c + 16, :])
                        nc.gpsimd.dma_start(cin[:, c * 512:(c + 1) * 512],
                                            s_loc[32 * c:32 * c + 16, :])
                    cout = trigger_ar((cin, [16, JD]))
                    s1 = fetch_ar(cout, 2)
                    broadcast_s([s1])
                else:
                    s_out = small.tile([128, 512], F32, tag="s_loc",
                                       name="s_out", bufs=2)
                    for c in range(4):
                        if c % 2 == 0:
                            nc.scalar.copy(s_out[32 * c:32 * c + 16, :],
                                           ps[32 * c:32 * c + 16, :])
                        else:
                            nc.vector.tensor_copy(s_out[32 * c:32 * c + 16, :],
                                                  ps[32 * c:32 * c + 16, :])
                        wdma_engines[c % 2].dma_start(
                            out_dram[:, c * 512:(c + 1) * 512],
                            s_out[32 * c:32 * c + 16, :])

    nc.compile()
    return nc


def pack_inputs(inp, W, b, n_cores: int, n_groups: int):
    """Host-side packing -> per-core in_maps. W columns in (d, j) order."""
    P = inp.shape[1]
    G = n_groups
    ploc = 8 * G
    npair = ploc // 2
    nblk = npair // 4
    assert n_cores * ploc == P

    bf = ml_dtypes.bfloat16
    if b is not None and np.any(b):
        raise NotImplementedError("nonzero bias b is not supported")
    # W[0]: [P, J, E, D] -> [P, E, (D, J)]
    Wt = np.ascontiguousarray(W[0].transpose(0, 2, 3, 1)).reshape(P, E, JD)
    Wp = Wt.reshape(P // 2, 2 * E, JD)
    Wb = Wp.reshape(n_cores, nblk, 4, 2 * E, JD).transpose(0, 1, 3, 2, 4)
    w_dev = np.ascontiguousarray(Wb).reshape(n_cores, nblk, 128, 4 * JD).astype(bf)

    # x: [B, P, E] -> block diag lhsT [c, 128, npair*32]
    inpT = inp.transpose(1, 2, 0)          # [P, E, B]
    arr = inpT.reshape(n_cores, npair, 2, E, B)
    x_dev = np.zeros((n_cores, 2, E, npair, 2, 16), np.float32)
    x_dev[:, 0, :, :, 0, :] = arr[:, :, 0].transpose(0, 2, 1, 3)
    x_dev[:, 1, :, :, 1, :] = arr[:, :, 1].transpose(0, 2, 1, 3)
    x_dev = x_dev.reshape(n_cores, 128, npair * 32).astype(bf)

    in_maps = []
    for c in range(n_cores):
        in_maps.append({"w": w_dev[c], "x": x_dev[c]})
    return in_maps


def squash_np(x):
    s2 = np.sum(x * x, axis=-1, keepdims=True)
    return x * (s2 / (1.0 + s2)) / np.sqrt(s2)


def unshard(results):
    """Combine per-core 'out' partials [128, 512] -> full output [B, J, D]."""
    s = np.zeros((16, JD), np.float64)
    for r in results:
        s += r["out"].astype(np.float64)
    v = squash_np(s.reshape(B, D, J).transpose(0, 2, 1))
    return v.astype(np.float32)


_CACHE = {}


def kernel(inp: np.ndarray, W: np.ndarray, b: np.ndarray) -> np.ndarray:
    n_cores, n_groups = 8, 16
    inp = np.asarray(inp, dtype=np.float32)
    W = np.asarray(W, dtype=np.float32)
    b = np.asarray(b, dtype=np.float32)

    key = (n_cores, n_groups)
    if key not in _CACHE:
        _CACHE[key] = build_program(n_cores, n_groups)
    nc = _CACHE[key]

    in_maps = pack_inputs(inp, W, b, n_cores, n_groups)
    res = run_bass_kernel_spmd(nc, in_maps, core_ids=list(range(n_cores)))
    return unshard(res.results)


# revision 4
# speedup vs baseline: 1.4111x; 1.4111x over previous
"""Trainium2 Bass kernel for CapsuleLayer dynamic routing.

Problem: u = einsum('bpe,pjed->bpjd', inp, W[0]) + b, then 3 routing
iterations (softmax over j, weighted sum over p, squash) -> vj [B,J,D].

Shapes: B=16, P=1024, J=32, Dp=D=64.  W is 512MB fp32 -> DMA dominated.

Strategy (8 NeuronCores):
 - Shard P across cores: 128 p's per core; all batches on every core.
 - Host packs W (bf16) in 2MiB blocks of 4 pairs: block g holds pairs
   4g..4g+3, each pair a [128=(2p x 64e), JD=(d,j)] panel.  lhsT[pair] =
   block-diag([inp_pe, inp_po]) so one matmul computes u for 2 capsules
   with full contraction rows, M=32; 4 pairs run concurrently in the 4
   PE column groups.
 - Routing on-device: agreement = per-g flat DVE mul (2x mode) + tree
   reduce; softmax without max subtraction; weighted p-sum via 0/1-mask
   (Delta) matmuls col-group-tiled 4-wide into PSUM accumulators at
   partition offsets 0/32/64/96; AllReduce (fp32) of the per-core sums.
 - AR plumbing: warmup AR triggered at t~0 absorbs the one-time CC
   barrier+ramp; s0 is split in two pieces (g<8 ships mid-phase-1);
   result fetches are SWDGE casts to bf16 ordered after all triggers so
   the gpsimd FIFO never blocks a doorbell.
 - Squash decoupled: the eye-matmul broadcasts RAW s (PSUM-accumulating
   the two s0 pieces), the squash scale f = scale^2*sqrt(t)/(1+scale^2 t)
   is computed concurrently (ACT square + halvings) and applied to the
   agreement output instead of to v.  Final iteration's partials summed
   + squashed on host.
"""

import numpy as np
import ml_dtypes

import concourse.bass as bass
import concourse.tile as tile
from concourse import bacc, mybir
from concourse.bass_utils import run_bass_kernel_spmd

F32 = mybir.dt.float32
BF16 = mybir.dt.bfloat16
AX = mybir.AxisListType
AF = mybir.ActivationFunctionType

B = 16      # batch
J = 32      # output capsules
D = 64      # output capsule dim
E = 64      # input capsule dim
JD = J * D  # 2048


def build_program(n_cores: int, n_groups: int):
    """Build the SPMD Bass program. Per core: P_loc = 8*n_groups capsules."""
    G = n_groups
    ploc = 8 * G
    npair = ploc // 2
    nblk = npair // 4          # 2MiB DMA blocks of 4 pairs (= 1 group)
    UFREE = G * JD             # u free elements per partition
    GS0 = 6                    # s0 split point (early AR over g < GS0)

    nc = bacc.Bacc("TRN2", target_bir_lowering=False, debug=False,
                   num_devices=n_cores)

    w_dram = nc.dram_tensor("w", [nblk, 128, 4 * JD], BF16, kind="ExternalInput")
    x_dram = nc.dram_tensor("x", [128, npair * 32], BF16, kind="ExternalInput")
    out_dram = nc.dram_tensor("out", [16, JD], F32, kind="ExternalOutput")

    with tile.TileContext(nc) as tc:
        with (
            tc.tile_pool(name="const", bufs=1) as constp,
            tc.tile_pool(name="wpool", bufs=2) as wpool,
            tc.tile_pool(name="upool", bufs=1) as upool,
            tc.tile_pool(name="work", bufs=2) as work,
            tc.tile_pool(name="small", bufs=1) as small,
            tc.tile_pool(name="pmain", bufs=2, space="PSUM") as pmain,
            tc.tile_pool(name="pacc", bufs=2, space="PSUM") as pacc,
            tc.tile_pool(name="dram", bufs=1, space="DRAM") as dramp,
        ):
            # ---- W tile for group 0 first (start streaming ASAP) ----
            wdma_engines = [nc.sync, nc.scalar]
            wtiles = {}

            def load_w(g):
                wt = wpool.tile([128, 4 * JD], BF16, tag="w",
                                name=f"wt{g}", bufs=3)
                wdma_engines[g % 2].dma_start(wt[:], w_dram[g])
                return wt

            wtiles[0] = load_w(0)
            wtiles[1] = load_w(1)

            # ---- static inputs -> SBUF ----
            x_sb = constp.tile([128, npair * 32], BF16)
            nc.sync.dma_start(x_sb[:], x_dram[:])
            # 0/1 mask constants built on-device (no DMA dependency):
            # delta[q, m] = (q % 16 == m); eye[r, q] = (q % 16 == r)
            I32 = mybir.dt.int32
            delta_sb = constp.tile([128, 16], BF16)
            qi = constp.tile([128, 128], I32)
            mi = constp.tile([128, 128], I32)
            ei = constp.tile([128, 128], F32)
            nc.gpsimd.iota(qi[:, :16], pattern=[[0, 16]], base=0,
                           channel_multiplier=1)
            nc.vector.tensor_scalar(qi[:, :16], qi[:, :16], 15, None,
                                    op0=mybir.AluOpType.bitwise_and)
            nc.gpsimd.iota(mi[:, :16], pattern=[[1, 16]], base=0,
                           channel_multiplier=0)
            nc.vector.tensor_tensor(ei[:, :16], qi[:, :16], mi[:, :16],
                                    op=mybir.AluOpType.is_equal)
            nc.vector.tensor_copy(delta_sb[:], ei[:, :16])
            eye_sb = constp.tile([16, 128], BF16)
            nc.gpsimd.iota(qi[:16, :], pattern=[[1, 128]], base=0,
                           channel_multiplier=0)
            nc.vector.tensor_scalar(qi[:16, :], qi[:16, :], 15, None,
                                    op0=mybir.AluOpType.bitwise_and)
            nc.gpsimd.iota(mi[:16, :], pattern=[[0, 128]], base=0,
                           channel_multiplier=1)
            nc.vector.tensor_tensor(ei[:16, :], qi[:16, :], mi[:16, :],
                                    op=mybir.AluOpType.is_equal)
            nc.vector.tensor_copy(eye_sb[:], ei[:16, :])

            u_sb = upool.tile([128, UFREE], BF16)

            # ---- collective helpers ----
            n_cc = [0]

            def trigger_ar(src_f32_dma_done, nelem=JD):
                """Issue the collective for a prepared cin; return cout."""
                i = n_cc[0]
                n_cc[0] += 1
                cin, shape = src_f32_dma_done
                cout = dramp.tile(shape, F32, tag=f"cout{i}",
                                  addr_space="Shared" if n_cores > 4 else "Local",
                                  name=f"cc_out{i}")
                nc.gpsimd.collective_compute(
                    "AllReduce", mybir.AluOpType.add,
                    replica_groups=[list(range(n_cores))],
                    ins=[cin.opt()], outs=[cout.opt()],
                )
                return cout

            # ---- phase 1: stream W, matmul u, evict, accumulate s0 ----
            # s0 accumulated in two psum pieces split at GS0 so the first
            # (ramp-paying) AllReduce hides under phase 1.  Delta matmuls
            # are col-group tiled: ns-chunk c accumulates at partition
            # offset 32c (4 concurrent PE column groups).
            state = {"ps": pacc.tile([128, 512], F32, tag="pacc", name="ps0_0"),
                     "started": set()}
            pend = []          # u_sb slabs whose delta matmuls are pending
            couts = []         # AR output dram tiles, in trigger order

            def flush_delta(draining):
                off, hn = pend.pop(0)
                ps = state["ps"]
                for t in range(2):
                    c = 2 * hn + t
                    start = c not in state["started"]
                    state["started"].add(c)
                    stop = draining and not any(h == hn for _, h in pend)
                    nc.tensor.matmul(
                        ps[32 * c:32 * c + 16, :],
                        delta_sb[:],
                        u_sb[:, off + t * 512: off + (t + 1) * 512],
                        tile_position=(0, 32 * c),
                        start=start, stop=stop,
                        skip_group_check=True,
                    )

            def ship_piece(idx):
                # drain pending slabs into this piece and ship its AllReduce
                while pend:
                    flush_delta(True)
                ps = state["ps"]
                s_loc = small.tile([128, 512], F32, tag="s_loc",
                                   name=f"sloc{idx}", bufs=2)
                cin = dramp.tile([16, JD], F32, tag=f"cin{idx}",
                                 name=f"cc_in{idx}")
                for c in range(4):
                    nc.scalar.copy(s_loc[32 * c:32 * c + 16, :],
                                   ps[32 * c:32 * c + 16, :])
                    nc.gpsimd.dma_start(cin[:, c * 512:(c + 1) * 512],
                                        s_loc[32 * c:32 * c + 16, :])
                couts.append(trigger_ar((cin, [16, JD])))
                state["started"] = set()

            for g in range(G):
                if g >= 1 and g + 1 < G:
                    wtiles[g + 1] = load_w(g + 1)
                if g == GS0:
                    ship_piece(0)
                    state["ps"] = pacc.tile([128, 512], F32, tag="pacc",
                                            name="ps0_1")
                wt = wtiles.pop(g)
                for hn in range(2):
                    pm = pmain.tile([128, 1024], F32, tag="pmain",
                                    name=f"pm{g}_{hn}")
                    # cg outer / ns inner: consecutive matmuls share lhsT so
                    # the weight load can be deduplicated.
                    for cg in range(4):
                        pi = 4 * g + cg
                        lhsT = x_sb[:, pi * 32:(pi + 1) * 32]
                        base = cg * JD + hn * 1024
                        for ns in range(2):
                            nc.tensor.matmul(
                                pm[32 * cg:32 * cg + 32, ns * 512:(ns + 1) * 512],
                                lhsT,
                                wt[:, base + ns * 512: base + (ns + 1) * 512],
                                tile_position=(0, 32 * cg),
                            )
                    off = g * JD + hn * 1024
                    # eviction on DVE (idle in phase 1) so the HWDGE queues
                    # only stream W at full rate
                    nc.vector.tensor_copy(u_sb[:, off:off + 1024], pm[:])
                    pend.append((off, hn))
                    if len(pend) > 2:
                        flush_delta(False)
            ship_piece(1)

            # fetch AR results AFTER all phase-1 triggers are enqueued so
            # the gpsimd FIFO never blocks a doorbell; SWDGE casts to bf16
            # during the fetch.
            def fetch_ar(cout, idx):
                dst = small.tile([16, JD], BF16, tag=f"ar{idx % 2}",
                                 name=f"ar_dst{idx}")
                nc.gpsimd.dma_start(dst[:], cout[:])
                return dst

            s0_a = fetch_ar(couts[0], 0)
            s0_b = fetch_ar(couts[1], 1)

            # ---- broadcast raw s to all 128 partitions (bf16) ----
            v_sb = constp.tile([128, JD], BF16)

            def broadcast_s(pieces):
                # v_sb[(k,b), col] = sum(piece[b, col] for piece in pieces)
                for hn in range(2):
                    pv = pmain.tile([128, 1024], F32, tag="pmain", name=f"pv{hn}")
                    for t in range(2):
                        for pi, piece in enumerate(pieces):
                            nc.tensor.matmul(
                                pv[:, t * 512:(t + 1) * 512], eye_sb[:],
                                piece[:, hn * 1024 + t * 512: hn * 1024 + (t + 1) * 512],
                                start=(pi == 0), stop=(pi == len(pieces) - 1),
                                skip_group_check=True,
                            )
                    nc.scalar.copy(v_sb[:, hn * 1024:(hn + 1) * 1024], pv[:])

            # ---- squash factor f = scale^2*sqrt(t) / (1 + scale^2*t) ----
            # computed from the RAW broadcast s in 128-partition space,
            # concurrent with the agreement muls (ACT + small DVE ops).
            def f_chain(scale, it):
                s2v = small.tile([128, JD], BF16, tag="s2v", name=f"s2v{it}")
                nc.vector.tensor_mul(s2v[:], v_sb[:], v_sb[:])
                n = JD // 2
                while n >= J * 2:
                    nc.vector.tensor_add(s2v[:, :n], s2v[:, :n], s2v[:, n:2 * n])
                    n //= 2
                t = small.tile([128, J], F32, tag="t", name=f"t{it}")
                nc.vector.tensor_add(t[:], s2v[:, :J], s2v[:, J:2 * J])
                st = small.tile([128, J], F32, tag="st", name=f"st{it}")
                nc.scalar.sqrt(st[:], t[:])
                den = small.tile([128, J], F32, tag="den", name=f"den{it}")
                nc.vector.tensor_scalar(den[:], t[:], scale * scale, 1.0,
                                        op0=mybir.AluOpType.mult,
                                        op1=mybir.AluOpType.add)
                rec = small.tile([128, J], F32, tag="rec", name=f"rec{it}")
                nc.vector.reciprocal(rec[:], den[:])
                f = small.tile([128, J], F32, tag="f", name=f"f{it}")
                nc.vector.scalar_tensor_tensor(f[:], st[:], scale * scale, rec[:],
                                               op0=mybir.AluOpType.mult,
                                               op1=mybir.AluOpType.mult)
                return f

            broadcast_s([s0_a, s0_b])

            # ---- routing iterations ----
            # Work split: DVE owns groups 0..12 (3 chunks); GpSimd
            # (measured ~0.31x of DVE throughput) owns groups 13-15.
            braw = constp.tile([128, G * J], F32)
            bij = constp.tile([128, G * J], F32)

            CHUNKS = [(0, 4, nc.vector), (4, 4, nc.vector), (8, 4, nc.vector),
                      (12, 4, nc.vector)]

            def agree_chunk(eng, g0, gc, b_h, tag):
                """tmp = u*v for groups [g0, g0+gc); in-place tree-reduce over
                d; write the agreement into b_h [128, gc*J] fp32."""
                ch = gc * JD
                u_ch = u_sb[:, g0 * JD:(g0 + gc) * JD]
                tmp = work.tile([128, ch], BF16, tag=f"tmp{tag}", name="tmp",
                                bufs=2 if tag == "v" else 1)
                eng.tensor_mul(
                    tmp[:].rearrange("p (g q) -> p g q", g=gc),
                    u_ch.rearrange("p (g q) -> p g q", g=gc),
                    v_sb[:].unsqueeze(1).broadcast_to([128, gc, JD]),
                )
                # pairwise add-tree over d, fully in-place inside tmp
                # (out == src0 range exactly at every level)
                r3 = tmp[:].rearrange("p (g q) -> p g q", g=gc)
                dl = D
                while dl > 2:
                    half = dl // 2 * J
                    eng.tensor_add(
                        r3[:, :, 0:half], r3[:, :, 0:half], r3[:, :, half:2 * half])
                    dl //= 2
                eng.tensor_add(
                    b_h.rearrange("p (g j) -> p g j", g=gc),
                    r3[:, :, 0:J], r3[:, :, J:2 * J])

            for it in (1, 2):
                scale = (1.0 / J) if it == 1 else 1.0
                for g0, gc, eng in CHUNKS:
                    tag = "g" if eng is nc.gpsimd else "v"
                    b_h = braw[:, g0 * J:(g0 + gc) * J]
                    agree_chunk(eng, g0, gc, b_h, tag=tag)
                f = f_chain(scale, it)
                fbc = f[:].unsqueeze(1).broadcast_to([128, G, J])
                braw3 = braw[:].rearrange("p (g j) -> p g j", g=G)
                if it == 1:
                    nc.vector.tensor_mul(
                        bij[:].rearrange("p (g j) -> p g j", g=G), braw3, fbc)
                else:
                    nc.vector.tensor_mul(braw3, braw3, fbc)
                    nc.vector.tensor_add(bij[:], bij[:], braw[:])
                # softmax over j (no max subtraction: |bij| is bounded small)
                eh = small.tile([128, G * J], F32, tag="eh", name="eh")
                nc.scalar.activation(eh[:], bij[:], AF.Exp)
                eh3 = eh[:].rearrange("p (g j) -> p g j", g=G)
                se = small.tile([128, G], F32, tag="se", name="se")
                nc.vector.reduce_sum(se[:], eh3, axis=AX.X)
                re = small.tile([128, G], F32, tag="re", name="re")
                nc.vector.reciprocal(re[:], se[:])
                c_full = small.tile([128, G * J], BF16, tag="c_h", name="c_full")
                nc.vector.tensor_mul(
                    c_full[:].rearrange("p (g j) -> p g j", g=G), eh3,
                    re[:].unsqueeze(2).broadcast_to([128, G, J]))
                ps = pacc.tile([128, 512], F32, tag="pacc", name=f"ps_it{it}")
                for g0, gc, eng in CHUNKS:
                    tag = "g" if eng is nc.gpsimd else "v"
                    ch = gc * JD
                    u_ch = u_sb[:, g0 * JD:(g0 + gc) * JD]
                    u4 = u_ch.rearrange("p (g d j) -> p g d j", g=gc, d=D)
                    # cu = u * c (c broadcast over middle d)
                    cu = work.tile([128, ch], BF16, tag=f"tmp{tag}", name="cu",
                                   bufs=2 if tag == "v" else 1)
                    eng.tensor_mul(
                        cu[:].rearrange("p (g d j) -> p g d j", g=gc, d=D),
                        u4,
                        c_full[:, g0 * J:(g0 + gc) * J]
                            .rearrange("p (g j) -> p g j", g=gc)
                            .unsqueeze(2).broadcast_to([128, gc, D, J]),
                    )
                    # s += sum_k cu  (Delta matmuls, col-group tiled 4-wide)
                    for gg in range(gc):
                        for c in range(4):
                            nc.tensor.matmul(
                                ps[32 * c:32 * c + 16, :],
                                delta_sb[:],
                                cu[:, gg * JD + c * 512: gg * JD + (c + 1) * 512],
                                tile_position=(0, 32 * c),
                                start=(g0 == 0 and gg == 0),
                                stop=(g0 == 12 and gg == gc - 1),
                                skip_group_check=True,
                            )
                if it == 1:
                    s_loc = small.tile([128, 512], F32, tag="s_loc",
                                       name="sloc_it1", bufs=2)
                    cin = dramp.tile([16, JD], F32, tag="cin_s1", name="cc_in_s1")
                    for c in range(4):
                        nc.scalar.copy(s_loc[32 * c:32 * c + 16, :],
                                       ps[32 * c:32 * c + 16, :])
                        nc.gpsimd.dma_start(cin[:, c * 512:(c + 1) * 512],
                                            s_loc[32 * c:32 * c + 16, :])
                    cout = trigger_ar((cin, [16, JD]))
                    s1 = fetch_ar(cout, 2)
                    broadcast_s([s1])
                else:
                    s_out = small.tile([128, 512], F32, tag="s_loc",
                                       name="s_out", bufs=2)
                    for c in range(4):
                        nc.scalar.copy(s_out[32 * c:32 * c + 16, :],
                                       ps[32 * c:32 * c + 16, :])
                        wdma_engines[c % 2].dma_start(
                            out_dram[:, c * 512:(c + 1) * 512],
                            s_out[32 * c:32 * c + 16, :])

    nc.compile()
    return nc


def pack_inputs(inp, W, b, n_cores: int, n_groups: int):
    """Host-side packing -> per-core in_maps. W columns in (d, j) order."""
    P = inp.shape[1]
    G = n_groups
    ploc = 8 * G
    npair = ploc // 2
    nblk = npair // 4
    assert n_cores * ploc == P

    bf = ml_dtypes.bfloat16
    if b is not None and np.any(b):
        raise NotImplementedError("nonzero bias b is not supported")
    # W[0]: [P, J, E, D] -> [P, E, (D, J)]
    Wt = np.ascontiguousarray(W[0].transpose(0, 2, 3, 1)).reshape(P, E, JD)
    Wp = Wt.reshape(P // 2, 2 * E, JD)
    Wb = Wp.reshape(n_cores, nblk, 4, 2 * E, JD).transpose(0, 1, 3, 2, 4)
    w_dev = np.ascontiguousarray(Wb).reshape(n_cores, nblk, 128, 4 * JD).astype(bf)

    # x: [B, P, E] -> block diag lhsT [c, 128, npair*32]
    inpT = inp.transpose(1, 2, 0)          # [P, E, B]
    arr = inpT.reshape(n_cores, npair, 2, E, B)
    x_dev = np.zeros((n_cores, 2, E, npair, 2, 16), np.float32)
    x_dev[:, 0, :, :, 0, :] = arr[:, :, 0].transpose(0, 2, 1, 3)
    x_dev[:, 1, :, :, 1, :] = arr[:, :, 1].transpose(0, 2, 1, 3)
    x_dev = x_dev.reshape(n_cores, 128, npair * 32).astype(bf)

    in_maps = []
    for c in range(n_cores):
        in_maps.append({"w": w_dev[c], "x": x_dev[c]})
    return in_maps


def squash_np(x):
    s2 = np.sum(x * x, axis=-1, keepdims=True)
    return x * (s2 / (1.0 + s2)) / np.sqrt(s2)


def unshard(results):
    """Combine per-core 'out' partials [128, 512] -> full output [B, J, D]."""
    s = np.zeros((16, JD), np.float64)
    for r in results:
        s += r["out"].astype(np.float64)
    v = squash_np(s.reshape(B, D, J).transpose(0, 2, 1))
    return v.astype(np.float32)


_CACHE = {}


def kernel(inp: np.ndarray, W: np.ndarray, b: np.ndarray) -> np.ndarray:
    n_cores, n_groups = 8, 16
    inp = np.asarray(inp, dtype=np.float32)
    W = np.asarray(W, dtype=np.float32)
    b = np.asarray(b, dtype=np.float32)

    key = (n_cores, n_groups)
    if key not in _CACHE:
        _CACHE[key] = build_program(n_cores, n_groups)
    nc = _CACHE[key]

    in_maps = pack_inputs(inp, W, b, n_cores, n_groups)
    res = run_bass_kernel_spmd(nc, in_maps, core_ids=list(range(n_cores)))
    return unshard(res.results)
